# revision 8
# baseline (speedup 1.0000x reference)
"""MetaOptNet SVM-head (QP interior point + query scoring) on 8 Trainium2 cores.

Math (validated against the jax reference in numpy simulation):
  Q = kron(M, I_10) with M = support@support.T + 10 I, so in class-major
  ordering H = Q + diag(lam/s) is block-diagonal: H_a = M + diag(d[:,a]).
  Per IPM iteration the Newton solves are applied through the structural
  Jacobi + one-Newton-Schulz form (E = offdiag(M) shared by all classes):
      H_a^-1 r ~= X1_a r = rdinv_a * (r - E (rdinv_a * r))
  and the Schur complement solve reduces to its diagonal (cond(S)~2, the
  fixed 15-iteration IPM is insensitive -- verified to 1e-4 output error).
  E@U products batch all 10 classes into one 4-instruction PE matmul group.

Sharding: QP state is tiny, so the full QP runs replicated on every core
(no collectives); the 8192 queries are split 8 ways for the scoring GEMM
(logits = query @ (support.T @ qp_sol)), log-softmax, and loss partials.

Layouts: 150-dim support vectors are stored [75 part, 2*10] (row-half r in
column block r) so one DVE op covers all classes and halves, and column
blocks serve directly as K-chunk operands of 75-contraction matmuls.
"""
import numpy as np
import ml_dtypes

N_WAY, N_SHOT = 10, 15
N_SUP = 150
HALF = 75
N_QUERY, D = 8192, 1024
C_REG = 0.1
MAX_ITER = 15
SIGMA = 0.1
EPS = 1e-30
NCORE = 8
NQ_SH = N_QUERY // NCORE          # 1024 queries per core
DCH = D // 128                    # 8 contraction chunks
QT = NQ_SH // 128                 # 8 query tiles per core

_CACHE = {}


def build_program(repeat=1):
    import concourse.tile as tile
    from concourse import bacc, mybir

    F32 = mybir.dt.float32
    BF16 = mybir.dt.bfloat16
    AX = mybir.AxisListType
    OP = mybir.AluOpType
    AF = mybir.ActivationFunctionType

    nc = bacc.Bacc("TRN2", target_bir_lowering=False, debug=False,
                   num_devices=NCORE)

    supT_d = nc.dram_tensor("supT", [D, N_SUP], F32, kind="ExternalInput").ap()
    supR_d = nc.dram_tensor("supR", [HALF, 2 * D], F32, kind="ExternalInput").ap()
    qh_d = nc.dram_tensor("qh", [D, NQ_SH], BF16, kind="ExternalInput").ap()
    ql_d = nc.dram_tensor("ql", [D, NQ_SH], BF16, kind="ExternalInput").ap()
    Ysup_d = nc.dram_tensor("Ysup", [HALF, 2 * N_WAY], F32, kind="ExternalInput").ap()
    eye75_d = nc.dram_tensor("eye75", [HALF, HALF], F32, kind="ExternalInput").ap()
    noteye_d = nc.dram_tensor("noteye75", [HALF, HALF], F32, kind="ExternalInput").ap()
    y1h_d = nc.dram_tensor("y1h", [128, QT * N_WAY], F32, kind="ExternalInput").ap()
    scale_d = nc.dram_tensor("scale", [1, 1], F32, kind="ExternalInput").ap()

    lp_d = nc.dram_tensor("lp", [128, QT * N_WAY], F32, kind="ExternalOutput").ap()
    loss_d = nc.dram_tensor("loss", [1, 1], F32, kind="ExternalOutput").ap()

    with tile.TileContext(nc) as tc:
        with tc.tile_pool(name="big", bufs=1) as big, \
             tc.tile_pool(name="state", bufs=1) as state, \
             tc.tile_pool(name="tmp", bufs=2) as tmp, \
             tc.tile_pool(name="tmps", bufs=2) as tmps, \
             tc.tile_pool(name="ps", bufs=2, space="PSUM") as ps, \
             tc.tile_pool(name="pssm", bufs=2, space="PSUM") as pssm, \
             tc.tile_pool(name="psK", bufs=2, space="PSUM") as psK:

            # ---- load inputs ----
            supT = big.tile([128, DCH, N_SUP], F32)
            nc.gpsimd.dma_start(out=supT[:], in_=supT_d.rearrange(
                "(c p) s -> p c s", p=128))
            supR = big.tile([HALF, 2, D], F32)
            nc.gpsimd.dma_start(out=supR[:], in_=supR_d.rearrange(
                "h (r d) -> h r d", r=2))
            qh = big.tile([128, DCH, NQ_SH], BF16)
            nc.gpsimd.dma_start(out=qh[:], in_=qh_d.rearrange(
                "(c p) q -> p c q", p=128))
            ql = big.tile([128, DCH, NQ_SH], BF16)
            nc.gpsimd.dma_start(out=ql[:], in_=ql_d.rearrange(
                "(c p) q -> p c q", p=128))
            Ysup = big.tile([HALF, 2 * N_WAY], F32)
            nc.gpsimd.dma_start(out=Ysup[:], in_=Ysup_d[:])
            eye75 = big.tile([HALF, HALF], F32)
            nc.gpsimd.dma_start(out=eye75[:], in_=eye75_d[:])
            noteye = big.tile([HALF, HALF], F32)
            nc.gpsimd.dma_start(out=noteye[:], in_=noteye_d[:])
            y1h = big.tile([128, QT * N_WAY], F32)
            nc.gpsimd.dma_start(out=y1h[:], in_=y1h_d[:])
            scale_sb = big.tile([1, 1], F32)
            nc.gpsimd.dma_start(out=scale_sb[:], in_=scale_d[:])

            ones75 = big.tile([HALF, 1], F32)
            nc.vector.memset(ones75[:], 1.0)
            onesr75 = big.tile([1, HALF], F32)
            nc.vector.memset(onesr75[:], 1.0)
            onesr128 = big.tile([1, 128], F32)
            nc.vector.memset(onesr128[:], 1.0)
            ones128 = big.tile([128, 1], F32)
            nc.vector.memset(ones128[:], 1.0)

            # persistent QP state
            Z = state.tile([HALF, 2 * N_WAY], F32)
            Sv = state.tile([HALF, 2 * N_WAY], F32)
            Lam = state.tile([HALF, 2 * N_WAY], F32)
            nu = state.tile([HALF, 2], F32)
            Ebf = state.tile([HALF, 4, HALF], BF16)
            dM = state.tile([HALF, 2], F32)
            W_sb = state.tile([128, DCH, N_WAY], F32)
            Wh = state.tile([128, DCH, N_WAY], BF16)
            Wl = state.tile([128, DCH, N_WAY], BF16)
            logits = state.tile([N_WAY, NQ_SH], F32)
            lpw = state.tile([128, QT, N_WAY], F32)

            def v3(t):   # view [75, 2*10] as [75, 2, 10]
                return t[:].rearrange("h (r a) -> h r a", r=2)

            for rep in range(repeat):
                # ================= K = sup@sup.T, E, dM =================
                Kps0 = psK.tile([HALF, N_SUP], F32, tag="KWL")
                Kps1 = psK.tile([HALF, N_SUP], F32, tag="KWL")
                Kps = [Kps0, Kps1]
                for m in range(2):
                    for c in range(DCH):
                        nc.tensor.matmul(
                            Kps[m][:],
                            supT[:, c, m * HALF:(m + 1) * HALF],
                            supT[:, c, :],
                            start=(c == 0), stop=(c == DCH - 1))
                for m in range(2):
                    # dM[:, m] = rowsum(K[mhalf, mhalf] * I) + 10
                    t75 = tmp.tile([HALF, HALF], F32, tag="t75")
                    nc.vector.tensor_tensor(
                        out=t75[:], in0=Kps[m][:, m * HALF:(m + 1) * HALF],
                        in1=eye75[:], op=OP.mult)
                    dcol = tmp.tile([HALF, 1], F32, tag="dcol")
                    nc.vector.tensor_reduce(
                        out=dcol[:], in_=t75[:], axis=AX.X, op=OP.add)
                    nc.vector.tensor_scalar_add(
                        out=dM[:, m:m + 1], in0=dcol[:], scalar1=10.0)
                for r in range(2):
                    for m in range(2):
                        if r == m:
                            nc.vector.tensor_tensor(
                                out=Ebf[:, r * 2 + m, :],
                                in0=Kps[r][:, m * HALF:(m + 1) * HALF],
                                in1=noteye[:], op=OP.mult)
                        else:
                            nc.vector.tensor_copy(
                                out=Ebf[:, r * 2 + m, :],
                                in_=Kps[r][:, m * HALF:(m + 1) * HALF])

                # ================= QP init =================
                # Z = 0.1*Y - 1 ; S = 1 ; Lam = 1 ; nu = 0
                nc.vector.tensor_scalar(
                    out=Z[:], in0=Ysup[:], scalar1=C_REG, scalar2=-1.0,
                    op0=OP.mult, op1=OP.add)
                nc.vector.memset(Sv[:], 1.0)
                nc.vector.memset(Lam[:], 1.0)
                nc.vector.memset(nu[:], 0.0)

                def batched_E(dst_ps, Ubf):
                    """dst_ps[75, 2*10] = E @ U for all classes (psum)."""
                    for m in range(2):
                        for r in range(2):
                            nc.tensor.matmul(
                                dst_ps[:, m * N_WAY:(m + 1) * N_WAY],
                                Ebf[:, r * 2 + m, :],
                                Ubf[:, r * N_WAY:(r + 1) * N_WAY],
                                start=(r == 0), stop=(r == 1))

                # ================= IPM iterations =================
                for it in range(MAX_ITER):
                    rs = tmp.tile([HALF, 2 * N_WAY], F32, tag="rs")
                    nc.vector.reciprocal(out=rs[:], in_=Sv[:])
                    hd = tmp.tile([HALF, 2 * N_WAY], F32, tag="hd")
                    nc.vector.tensor_tensor(out=hd[:], in0=Lam[:], in1=rs[:],
                                            op=OP.mult)
                    nc.vector.tensor_tensor(
                        out=v3(hd), in0=v3(hd),
                        in1=dM[:].unsqueeze(2).broadcast_to([HALF, 2, N_WAY]),
                        op=OP.add)
                    rdinv = tmp.tile([HALF, 2 * N_WAY], F32, tag="rdinv")
                    nc.vector.reciprocal(out=rdinv[:], in_=hd[:])

                    # mu and Rc
                    prod = tmp.tile([HALF, 2 * N_WAY], F32, tag="prod")
                    nc.vector.tensor_tensor(out=prod[:], in0=Sv[:], in1=Lam[:],
                                            op=OP.mult)
                    pr1 = tmp.tile([HALF, 1], F32, tag="pr1")
                    nc.vector.tensor_reduce(out=pr1[:], in_=prod[:], axis=AX.X,
                                            op=OP.add)
                    mups = pssm.tile([1, 1], F32, tag="psm")
                    nc.tensor.matmul(mups[:], ones75[:], pr1[:], start=True,
                                     stop=True)
                    sigmu = tmps.tile([1, 1], F32, tag="sigmu")
                    nc.vector.tensor_scalar_mul(
                        out=sigmu[:], in0=mups[:],
                        scalar1=SIGMA / (N_SUP * N_WAY))
                    sigbc = pssm.tile([HALF, 1], F32, tag="psm")
                    nc.tensor.matmul(sigbc[:], onesr75[:], sigmu[:],
                                     start=True, stop=True)
                    Rc = tmp.tile([HALF, 2 * N_WAY], F32, tag="Rc")
                    nc.vector.tensor_scalar(
                        out=Rc[:], in0=prod[:], scalar1=sigbc[:],
                        scalar2=None, op0=OP.subtract)

                    # R1 = Rc*rs - E@Z - dM*Z + Y - Lam - nu
                    Zb = tmp.tile([HALF, 2 * N_WAY], BF16, tag="Zb")
                    nc.vector.tensor_copy(out=Zb[:], in_=Z[:])
                    EZ = ps.tile([HALF, 2 * N_WAY], F32, tag="EU")
                    batched_E(EZ, Zb)
                    R1 = tmp.tile([HALF, 2 * N_WAY], F32, tag="R1")
                    nc.vector.tensor_tensor(out=R1[:], in0=Rc[:], in1=rs[:],
                                            op=OP.mult)
                    nc.vector.tensor_tensor(out=R1[:], in0=R1[:], in1=EZ[:],
                                            op=OP.subtract)
                    mdz = tmp.tile([HALF, 2 * N_WAY], F32, tag="mdz")
                    nc.vector.tensor_tensor(
                        out=v3(mdz), in0=v3(Z),
                        in1=dM[:].unsqueeze(2).broadcast_to([HALF, 2, N_WAY]),
                        op=OP.mult)
                    nc.vector.tensor_tensor(out=R1[:], in0=R1[:], in1=mdz[:],
                                            op=OP.subtract)
                    nc.vector.tensor_tensor(out=R1[:], in0=R1[:], in1=Lam[:],
                                            op=OP.subtract)
                    nc.vector.tensor_tensor(out=R1[:], in0=R1[:], in1=Ysup[:],
                                            op=OP.add)
                    nc.vector.tensor_tensor(
                        out=v3(R1), in0=v3(R1),
                        in1=nu[:].unsqueeze(2).broadcast_to([HALF, 2, N_WAY]),
                        op=OP.subtract)

                    # X = rdinv*(R1 - E@(rdinv*R1))
                    U1 = tmp.tile([HALF, 2 * N_WAY], BF16, tag="U1")
                    nc.vector.tensor_tensor(out=U1[:], in0=rdinv[:], in1=R1[:],
                                            op=OP.mult)
                    EU1 = ps.tile([HALF, 2 * N_WAY], F32, tag="EU")
                    batched_E(EU1, U1)
                    X = tmp.tile([HALF, 2 * N_WAY], F32, tag="X")
                    nc.vector.tensor_tensor(out=X[:], in0=R1[:], in1=EU1[:],
                                            op=OP.subtract)
                    nc.vector.tensor_tensor(out=X[:], in0=X[:], in1=rdinv[:],
                                            op=OP.mult)

                    # dnu = (rowsum_a(X) + rowsum_a(Z)) / rowsum_a(rdinv)
                    sx = tmps.tile([HALF, 2], F32, tag="sx")
                    nc.vector.tensor_reduce(out=sx[:], in_=v3(X), axis=AX.X,
                                            op=OP.add)
                    rp = tmps.tile([HALF, 2], F32, tag="rp")
                    nc.vector.tensor_reduce(out=rp[:], in_=v3(Z), axis=AX.X,
                                            op=OP.add)
                    nc.vector.tensor_tensor(out=sx[:], in0=sx[:], in1=rp[:],
                                            op=OP.add)
                    gd = tmps.tile([HALF, 2], F32, tag="gd")
                    nc.vector.tensor_reduce(out=gd[:], in_=v3(rdinv), axis=AX.X,
                                            op=OP.add)
                    gi = tmps.tile([HALF, 2], F32, tag="gi")
                    nc.vector.reciprocal(out=gi[:], in_=gd[:])
                    dnu = tmps.tile([HALF, 2], F32, tag="dnu")
                    nc.vector.tensor_tensor(out=dnu[:], in0=sx[:], in1=gi[:],
                                            op=OP.mult)

                    # Wv = rdinv*(dnu - E@(rdinv*dnu))
                    U2 = tmp.tile([HALF, 2 * N_WAY], BF16, tag="U2")
                    nc.vector.tensor_tensor(
                        out=v3(U2), in0=v3(rdinv),
                        in1=dnu[:].unsqueeze(2).broadcast_to([HALF, 2, N_WAY]),
                        op=OP.mult)
                    EU2 = ps.tile([HALF, 2 * N_WAY], F32, tag="EU")
                    batched_E(EU2, U2)
                    Wv = tmp.tile([HALF, 2 * N_WAY], F32, tag="Wv")
                    nc.vector.tensor_tensor(
                        out=v3(Wv),
                        in0=dnu[:].unsqueeze(2).broadcast_to([HALF, 2, N_WAY]),
                        in1=v3(EU2), op=OP.subtract)
                    nc.vector.tensor_tensor(out=Wv[:], in0=Wv[:], in1=rdinv[:],
                                            op=OP.mult)

                    dZ = tmp.tile([HALF, 2 * N_WAY], F32, tag="dZ")
                    nc.vector.tensor_tensor(out=dZ[:], in0=X[:], in1=Wv[:],
                                            op=OP.subtract)
                    # ndLam = (Rc - Lam*dZ)*rs  (= -dLam)
                    ndl = tmp.tile([HALF, 2 * N_WAY], F32, tag="ndl")
                    nc.vector.tensor_tensor(out=ndl[:], in0=Lam[:], in1=dZ[:],
                                            op=OP.mult)
                    nc.vector.tensor_tensor(out=ndl[:], in0=Rc[:], in1=ndl[:],
                                            op=OP.subtract)
                    nc.vector.tensor_tensor(out=ndl[:], in0=ndl[:], in1=rs[:],
                                            op=OP.mult)

                    # alpha = min(1, 0.99*min(S/max(dZ,eps), Lam/max(ndl,eps)))
                    rat = tmp.tile([HALF, 2 * N_WAY], F32, tag="rat")
                    nc.vector.tensor_scalar_max(out=rat[:], in0=dZ[:],
                                                scalar1=EPS)
                    nc.vector.reciprocal(out=rat[:], in_=rat[:])
                    nc.vector.tensor_tensor(out=rat[:], in0=rat[:], in1=Sv[:],
                                            op=OP.mult)
                    rmins = tmps.tile([HALF, 1], F32, tag="rmins")
                    nc.vector.tensor_reduce(out=rmins[:], in_=rat[:], axis=AX.X,
                                            op=OP.min)
                    rat2 = tmp.tile([HALF, 2 * N_WAY], F32, tag="rat2")
                    nc.vector.tensor_scalar_max(out=rat2[:], in0=ndl[:],
                                                scalar1=EPS)
                    nc.vector.reciprocal(out=rat2[:], in_=rat2[:])
                    nc.vector.tensor_tensor(out=rat2[:], in0=rat2[:],
                                            in1=Lam[:], op=OP.mult)
                    rminl = tmps.tile([HALF, 1], F32, tag="rminl")
                    nc.vector.tensor_reduce(out=rminl[:], in_=rat2[:],
                                            axis=AX.X, op=OP.min)
                    nc.vector.tensor_tensor(out=rmins[:], in0=rmins[:],
                                            in1=rminl[:], op=OP.min)
                    rtp = pssm.tile([1, HALF], F32, tag="psm")
                    nc.tensor.transpose(rtp[:], rmins[:], eye75[:])
                    amin = tmps.tile([1, 1], F32, tag="amin")
                    nc.vector.tensor_reduce(out=amin[:], in_=rtp[:], axis=AX.X,
                                            op=OP.min)
                    alpha = tmps.tile([1, 1], F32, tag="alpha")
                    nc.vector.tensor_scalar(
                        out=alpha[:], in0=amin[:], scalar1=0.99, scalar2=1.0,
                        op0=OP.mult, op1=OP.min)
                    abc = pssm.tile([HALF, 1], F32, tag="psm")
                    nc.tensor.matmul(abc[:], onesr75[:], alpha[:], start=True,
                                     stop=True)

                    # updates
                    m1 = tmp.tile([HALF, 2 * N_WAY], F32, tag="m1")
                    nc.vector.tensor_scalar(
                        out=m1[:], in0=dZ[:], scalar1=abc[:], scalar2=None,
                        op0=OP.mult)
                    nc.vector.tensor_tensor(out=Z[:], in0=Z[:], in1=m1[:],
                                            op=OP.add)
                    nc.vector.tensor_tensor(out=Sv[:], in0=Sv[:], in1=m1[:],
                                            op=OP.subtract)
                    nc.vector.tensor_scalar(
                        out=m1[:], in0=ndl[:], scalar1=abc[:], scalar2=None,
                        op0=OP.mult)
                    nc.vector.tensor_tensor(out=Lam[:], in0=Lam[:], in1=m1[:],
                                            op=OP.subtract)
                    m3 = tmps.tile([HALF, 2], F32, tag="m3")
                    nc.vector.tensor_scalar(
                        out=m3[:], in0=dnu[:], scalar1=abc[:], scalar2=None,
                        op0=OP.mult)
                    nc.vector.tensor_tensor(out=nu[:], in0=nu[:], in1=m3[:],
                                            op=OP.add)

                # ================= W = scale * sup.T @ Z =================
                scbc = pssm.tile([128, 1], F32, tag="psm")
                nc.tensor.matmul(scbc[:], onesr128[:], scale_sb[:], start=True,
                                 stop=True)
                for c in range(DCH):
                    Wps = psK.tile([128, N_WAY], F32, tag="KWL")
                    for r in range(2):
                        nc.tensor.matmul(
                            Wps[:], supR[:, r, c * 128:(c + 1) * 128],
                            Z[:, r * N_WAY:(r + 1) * N_WAY],
                            start=(r == 0), stop=(r == 1))
                    nc.vector.tensor_scalar(
                        out=W_sb[:, c, :], in0=Wps[:], scalar1=scbc[:],
                        scalar2=None, op0=OP.mult)
                nc.vector.tensor_copy(out=Wh[:], in_=W_sb[:])
                nc.vector.tensor_tensor(out=Wl[:], in0=W_sb[:], in1=Wh[:],
                                        op=OP.subtract)

                # ============ logits.T = W.T @ qT  (bf16 hi/lo) ============
                for nq in range(2):
                    lps = psK.tile([N_WAY, 512], F32, tag="KWL")
                    first = True
                    for c in range(DCH):
                        for A, B in ((Wh, qh), (Wh, ql), (Wl, qh)):
                            nc.tensor.matmul(
                                lps[:], A[:, c, :],
                                B[:, c, nq * 512:(nq + 1) * 512],
                                start=first,
                                stop=(c == DCH - 1 and A is Wl))
                            first = False
                    nc.vector.tensor_copy(
                        out=logits[:, nq * 512:(nq + 1) * 512], in_=lps[:])

                # ============ transpose to [q,10], log-softmax ============
                for j in range(QT):
                    trp = pssm.tile([128, N_WAY], F32, tag="psm")
                    nc.tensor.transpose(
                        trp[:], logits[:, j * 128:(j + 1) * 128],
                        eye75[0:N_WAY, 0:N_WAY])
                    nc.vector.tensor_copy(out=lpw[:, j, :], in_=trp[:])
                mx = tmps.tile([128, QT], F32, tag="mx")
                nc.vector.tensor_reduce(out=mx[:], in_=lpw[:], axis=AX.X,
                                        op=OP.max)
                nc.vector.tensor_tensor(
                    out=lpw[:],
                    in0=lpw[:],
                    in1=mx[:].unsqueeze(2).broadcast_to([128, QT, N_WAY]),
                    op=OP.subtract)
                ex = tmp.tile([128, QT, N_WAY], F32, tag="ex")
                nc.scalar.activation(out=ex[:], in_=lpw[:], func=AF.Exp)
                sm = tmps.tile([128, QT], F32, tag="sm")
                nc.vector.tensor_reduce(out=sm[:], in_=ex[:], axis=AX.X,
                                        op=OP.add)
                lg = tmps.tile([128, QT], F32, tag="lg")
                nc.scalar.activation(out=lg[:], in_=sm[:], func=AF.Ln)
                nc.vector.tensor_tensor(
                    out=lpw[:],
                    in0=lpw[:],
                    in1=lg[:].unsqueeze(2).broadcast_to([128, QT, N_WAY]),
                    op=OP.subtract)

                # ============ loss partial = sum(y1h * lp) ============
                ml = tmp.tile([128, QT * N_WAY], F32, tag="ml")
                nc.vector.tensor_tensor(
                    out=ml[:], in0=lpw[:].rearrange("p q a -> p (q a)"),
                    in1=y1h[:], op=OP.mult)
                lred = tmps.tile([128, 1], F32, tag="lred")
                nc.vector.tensor_reduce(out=lred[:], in_=ml[:], axis=AX.X,
                                        op=OP.add)
                lossps = pssm.tile([1, 1], F32, tag="psm")
                nc.tensor.matmul(lossps[:], ones128[:], lred[:], start=True,
                                 stop=True)
                loss_sb = tmps.tile([1, 1], F32, tag="loss_sb")
                nc.vector.tensor_copy(out=loss_sb[:], in_=lossps[:])

                # ============ outputs ============
                nc.gpsimd.dma_start(
                    out=lp_d[:], in_=lpw[:].rearrange("p q a -> p (q a)"))
                nc.gpsimd.dma_start(out=loss_d[:], in_=loss_sb[:])

    nc.compile()
    return nc


class _Runner:
    def __init__(self, nc, n_cores):
        import jax
        import numpy as _np
        from jax.sharding import Mesh, PartitionSpec
        from jax.experimental.shard_map import shard_map
        import concourse.mybir as mybir
        from concourse.bass2jax import (
            _bass_exec_p, install_neuronx_cc_hook, partition_id_tensor)

        install_neuronx_cc_hook()
        self.n_cores = n_cores
        self.partition_name = (
            nc.partition_id_tensor.name if nc.partition_id_tensor else None)
        in_names, out_names, out_avals, zero_outs = [], [], [], []
        for alloc in nc.m.functions[0].allocations:
            if not isinstance(alloc, mybir.MemoryLocationSet):
                continue
            name = alloc.memorylocations[0].name
            if alloc.kind == "ExternalInput":
                if name != self.partition_name:
                    in_names.append(name)
            elif alloc.kind == "ExternalOutput":
                out_names.append(name)
                shape = tuple(alloc.tensor_shape)
                dtype = mybir.dt.np(alloc.dtype)
                out_avals.append(jax.core.ShapedArray(shape, dtype))
                zero_outs.append(_np.zeros(shape, dtype))
        self.n_params = len(in_names)
        self.in_names = in_names + out_names
        self.out_names = out_names
        self.out_avals = out_avals
        self.zero_outs = zero_outs
        if self.partition_name is not None:
            self.in_names.append(self.partition_name)
        out_avals_t = tuple(out_avals)
        in_names_t = tuple(self.in_names)
        out_names_t = tuple(out_names)

        def _body(*args):
            operands = list(args)
            if self.partition_name is not None:
                operands.append(partition_id_tensor())
            return tuple(_bass_exec_p.bind(
                *operands, out_avals=out_avals_t, in_names=in_names_t,
                out_names=out_names_t, lowering_input_output_aliases=(),
                sim_require_finite=True, sim_require_nnan=True, nc=nc))

        devices = jax.devices()[:n_cores]
        mesh = Mesh(_np.asarray(devices), ("core",))
        in_specs = (PartitionSpec("core"),) * (self.n_params + len(out_names))
        out_specs = (PartitionSpec("core"),) * len(out_names)
        self.fn = jax.jit(
            shard_map(_body, mesh=mesh, in_specs=in_specs,
                      out_specs=out_specs, check_rep=False),
            keep_unused=True)
        self._jax = jax

    def run(self, in_maps):
        np_ = np
        per_core = [
            [np_.ascontiguousarray(m[name]) for name in
             self.in_names[:self.n_params]]
            for m in in_maps
        ]
        concat_in = [
            np_.concatenate([per_core[c][i] for c in range(self.n_cores)],
                            axis=0)
            for i in range(self.n_params)
        ]
        concat_zeros = [
            np_.zeros((self.n_cores * z.shape[0], *z.shape[1:]), z.dtype)
            for z in self.zero_outs
        ]
        out = self.fn(*concat_in, *concat_zeros)
        self._jax.block_until_ready(out)
        return [
            {
                name: np_.asarray(out[i]).reshape(
                    self.n_cores, *self.out_avals[i].shape)[c]
                for i, name in enumerate(self.out_names)
            }
            for c in range(self.n_cores)
        ]


def _prepare_inputs(feat, label_support, label_query, scale):
    feat = np.ascontiguousarray(np.asarray(feat, dtype=np.float32))
    sup = feat[:N_SUP]
    sup_labels = np.repeat(np.asarray(label_support).astype(np.int64), N_SHOT)
    Ysup = np.zeros((N_SUP, N_WAY), np.float32)
    Ysup[np.arange(N_SUP), sup_labels] = 1.0
    # [75, 2, 10] -> [75, 20] (row-half r in column block r)
    Ysup_t = np.ascontiguousarray(
        Ysup.reshape(2, HALF, N_WAY).transpose(1, 0, 2).reshape(HALF, 2 * N_WAY))
    supT = np.ascontiguousarray(sup.T)                       # [1024, 150]
    supR = np.ascontiguousarray(
        sup.reshape(2, HALF, D).transpose(1, 0, 2).reshape(HALF, 2 * D))
    eye75 = np.eye(HALF, dtype=np.float32)
    noteye = 1.0 - eye75
    scale_in = np.asarray(scale, np.float32).reshape(1, 1)
    lq = np.asarray(label_query).astype(np.int64)

    in_maps = []
    for c in range(NCORE):
        qs = feat[N_SUP + c * NQ_SH: N_SUP + (c + 1) * NQ_SH]   # [1024, 1024]
        qT = np.ascontiguousarray(qs.T)                         # [d, q]
        qhn = qT.astype(ml_dtypes.bfloat16)
        qln = (qT - qhn.astype(np.float32)).astype(ml_dtypes.bfloat16)
        lqs = lq[c * NQ_SH:(c + 1) * NQ_SH]
        y1h = np.zeros((NQ_SH, N_WAY), np.float32)
        y1h[np.arange(NQ_SH), lqs] = 1.0
        # device layout [128 part, QT, 10]: query index = j*128 + p
        y1h_t = np.ascontiguousarray(
            y1h.reshape(QT, 128, N_WAY).transpose(1, 0, 2).reshape(
                128, QT * N_WAY))
        in_maps.append({
            "supT": supT, "supR": supR, "qh": qhn, "ql": qln,
            "Ysup": Ysup_t, "eye75": eye75, "noteye75": noteye,
            "y1h": y1h_t, "scale": scale_in,
        })
    return in_maps


def _assemble(results):
    lp_full = np.empty((N_QUERY, N_WAY), np.float32)
    total = 0.0
    for c in range(NCORE):
        arr = results[c]["lp"].reshape(128, QT, N_WAY).transpose(1, 0, 2)
        lp_full[c * NQ_SH:(c + 1) * NQ_SH] = arr.reshape(NQ_SH, N_WAY)
        total += float(results[c]["loss"][0, 0])
    loss = np.float32(-total / N_QUERY)
    return lp_full, loss


def get_runner(repeat=1):
    key = ("runner", repeat)
    if key not in _CACHE:
        nc = build_program(repeat=repeat)
        _CACHE[key] = _Runner(nc, NCORE)
    return _CACHE[key]


def kernel(feat, label_support, label_query, scale):
    runner = get_runner()
    in_maps = _prepare_inputs(feat, label_support, label_query, scale)
    results = runner.run(in_maps)
    return _assemble(results)
